# revision 15
# baseline (speedup 1.0000x reference)
"""Trainium2 Bass kernel for the AllGroupsExpertRunner MoE problem.

Math (dense-masked reference):
    x = tokens.reshape(M, D)                                # M = B*N = 8192
    out = sum_e w[:, e] * (gelu(x @ Wg[e]) * (x @ Wv[e])) @ Wo[e] * scales[e]
    where w = where(dispatch > 0, combine, 0)

Only tokens with w[:, e] > 0 matter for expert e. Sharding: two-segment
expert parallelism. Each core runs the same SPMD program over NT = T*128
token slots split at a fixed boundary NTA = F*128 into segments A and B,
each with its own full weight set. The host bin-packs the experts'
routed-token tiles into the 16 (core, segment) slots, so heavy experts
spill across cores and the per-core token count is ~total/8 instead of
max-per-expert (2176 vs 2304 for the reference routing). Routing weights
and output scales are applied on the host during the scatter-add.

All matmul operands are bf16 (fp32 PSUM accumulation): 1 row/cycle on the
PE, half the HBM traffic of fp32. Measured rel err ~4e-3 vs the 2e-2 gate.

DMA scheduling notes (from trace analysis): each engine has only 4 DMA
completion semaphores, so the 5th dma_start on an engine blocks until the
1st fully completes. The scalar engine must run gelus from ~t+15us, so it
gets at most 4 up-front DMAs; its later weight loads are injected between
gelus where the completion waits are free. Queue FIFO delivery (~200GB/s)
paces the start, so the first-needed wgA tiles are quarter tiles right
behind x chunk 0 on the sync queue, and wvA h-half tiles load on the
scalar queue in parallel. Wo / late x / output tiles ride the gpsimd
SWDGE queue (~70GB/s), which has no compute to block.

Per-core program, per token chunk (<=512):
  stage A, per 128-wide H block:
      g^T = Wg_blk^T @ xT-chunk   (4 accumulating matmuls over D)
      v^T = Wv_blk^T @ xT-chunk
      hT_blk = gelu(g^T) * v^T    (ACT + DVE, bf16 out)
  stage B (emitted one chunk late so hT is long since ready), per
  128-token tile:
      out_tile = hT^T @ Wo        (16 accumulating matmuls over H)
      DVE copy PSUM -> bf16, DMA out on the gpsimd queue.
"""

import numpy as np
import ml_dtypes

D = 512
H = 2048
E = 8
P = 128
MT = 512  # max token chunk (PSUM bank = 512 fp32)
ND = D // P  # 4 k-tiles over D
NH = H // P  # 16 k-tiles over H

_CACHE: dict = {}

BF16 = ml_dtypes.bfloat16


def _chunk_sizes(n):
    out = [MT] * (n // MT)
    if n % MT:
        out.append(n % MT)
    return out


def _build_program(NT: int, NTA: int):
    from contextlib import ExitStack

    import concourse.bacc as bacc
    import concourse.tile as tile
    import concourse.mybir as mybir
    import concourse.bass as bass_mod

    assert NT % P == 0 and NTA % P == 0 and 0 < NTA <= NT
    f32 = mybir.dt.float32
    BF = mybir.dt.bfloat16

    nc = bacc.Bacc("TRN2", target_bir_lowering=False, debug=False)

    xp = nc.dram_tensor("xp", [D * NT], BF, kind="ExternalInput")
    wts = {}
    for s in ("a", "b"):
        wts[s] = {
            "wg": nc.dram_tensor(f"wg_{s}", [D, H], BF, kind="ExternalInput"),
            "wv": nc.dram_tensor(f"wv_{s}", [D, H], BF, kind="ExternalInput"),
            # woP: host-packed [NH//4, 128, 4*D]: tile j row p holds
            # wo[(4j+k)*128+p, :] for k=0..3 -> 4KB DMA rows
            "wo": nc.dram_tensor(f"wo_{s}", [NH // 4 * P * 4 * D], BF,
                                 kind="ExternalInput"),
        }
    out = nc.dram_tensor("out", [NT, D], BF, kind="ExternalOutput")

    # chunk list: (token_offset, size, segment)
    chunks = []
    t0 = 0
    for mt in _chunk_sizes(NTA):
        chunks.append((t0, mt, "a"))
        t0 += mt
    for mt in _chunk_sizes(NT - NTA):
        chunks.append((t0, mt, "b"))
        t0 += mt

    gelu = mybir.ActivationFunctionType.Gelu

    with tile.TileContext(nc) as tc, ExitStack() as ctx:
        wpool = ctx.enter_context(tc.tile_pool(name="w", bufs=1))
        xpool = ctx.enter_context(tc.tile_pool(name="x", bufs=1))
        hpool = ctx.enter_context(tc.tile_pool(name="h", bufs=3))
        gpool = ctx.enter_context(tc.tile_pool(name="g", bufs=4))
        opool = ctx.enter_context(tc.tile_pool(name="o", bufs=6))
        psg = ctx.enter_context(tc.tile_pool(name="psg", bufs=3, space="PSUM"))
        psv = ctx.enter_context(tc.tile_pool(name="psv", bufs=3, space="PSUM"))
        pso = ctx.enter_context(tc.tile_pool(name="pso", bufs=2, space="PSUM"))

        # weight tiles: wg segment A in 512-col quarters (one DMA each, so
        # the first matmul only waits on x + one 128KB tile), wg segment B
        # whole [128, 2048] tiles, wv both segments in 1024-col halves
        # (loaded on the scalar queue in 4-DMA batches), wo in 4-hslice
        # merged tiles (host-packed for 4KB DMA rows).
        QW, HW_ = 512, 1024
        wgq = {"a": [[wpool.tile([P, QW], BF, tag=f"wga{d}q{q}", name=f"wga{d}q{q}")
                      for q in range(H // QW)] for d in range(ND)]}
        wgw = {"b": [wpool.tile([P, H], BF, tag=f"wgb{d}", name=f"wgb{d}")
                     for d in range(ND)]}
        wvq = {"a": [[wpool.tile([P, QW], BF, tag=f"wva{d}q{q}", name=f"wva{d}q{q}")
                      for q in range(H // QW)] for d in range(ND)]}
        wvh = {"b": [[wpool.tile([P, HW_], BF, tag=f"wvb{d}f{f}", name=f"wvb{d}f{f}")
                      for f in range(H // HW_)] for d in range(ND)]}
        wot = {s: [wpool.tile([P, 4 * D], BF, tag=f"wo{s}{j}", name=f"wo{s}{j}")
                   for j in range(NH // 4)] for s in ("a", "b")}

        def wg_ap(s, d, h):
            if s == "a":
                q, c = divmod(h * P, QW)
                return wgq["a"][d][q][:, c:c + P]
            return wgw["b"][d][:, h * P:(h + 1) * P]

        def wv_ap(s, d, h):
            if s == "a":
                q, c = divmod(h * P, QW)
                return wvq["a"][d][q][:, c:c + P]
            f, c = divmod(h * P, HW_)
            return wvh["b"][d][f][:, c:c + P]

        # x chunk tiles: [128, ND, mt], host-packed so row p holds
        # xT[d*128+p, tok0:tok0+mt] for d=0..3 (4KB rows at mt=512)
        xq = []
        xoff = []
        off = 0
        for (tok0, mt, s) in chunks:
            xq.append(xpool.tile([P, ND, mt], BF, tag=f"xq{tok0}",
                                 name=f"xq{tok0}"))
            xoff.append(off)
            off += P * ND * mt

        def xp_ap(ci):
            tok0, mt, _ = chunks[ci]
            return bass_mod.AP(tensor=xp, offset=xoff[ci],
                               ap=[[ND * mt, P], [1, ND * mt]])

        def wo_ap(s, j):
            return bass_mod.AP(tensor=wts[s]["wo"], offset=j * P * 4 * D,
                               ap=[[4 * D, P], [1, 4 * D]])

        ca = [ci for ci, c in enumerate(chunks) if c[2] == "a"]
        cb = [ci for ci, c in enumerate(chunks) if c[2] == "b"]

        # act-table preload: a dummy gelu on a zeroed scratch tile makes the
        # scalar engine pull the gelu table during the DMA warmup instead of
        # right before the first real gelu.
        # scalar queue: exactly its 4 free up-front DMA slots carry wv q0 in
        # parallel with sync's x+wg q0, pulling the first v-matmul earlier.
        for d in range(ND):
            nc.scalar.dma_start(
                out=wvq["a"][d][0][:],
                in_=wts["a"]["wv"][d * P:(d + 1) * P, 0:QW])
        scratch = gpool.tile([P, 8], f32, tag="scratch", name="scratch")
        nc.vector.memset(scratch[:], 0.0)
        nc.scalar.activation(scratch[:], scratch[:], gelu)

        # --- sync queue, strict demand order: x chunk 0 whole (4KB rows run
        # at full packet rate even on the cold queue), then wg/wv quarter
        # tiles interleaved in h-consumption order, then segment-A Wo, later
        # x chunks, and the segment-B bulk. Output DMAs are appended by
        # emit_B behind these. Keeping the critical stream on one queue in
        # exact consumption order measured better than spreading it across
        # the cold scalar/gpsimd queues.
        nc.sync.dma_start(out=xq[ca[0]][:], in_=xp_ap(ca[0]))
        for q in range(H // QW):
            for d in range(ND):
                nc.sync.dma_start(
                    out=wgq["a"][d][q][:],
                    in_=wts["a"]["wg"][d * P:(d + 1) * P, q * QW:(q + 1) * QW])
            if q == 0:
                continue  # wv q0 rides the scalar queue in parallel
            for d in range(ND):
                nc.sync.dma_start(
                    out=wvq["a"][d][q][:],
                    in_=wts["a"]["wv"][d * P:(d + 1) * P, q * QW:(q + 1) * QW])
        for j in (0, 1):
            nc.sync.dma_start(out=wot["a"][j][:], in_=wo_ap("a", j))
        for ci in ca[1:]:
            nc.sync.dma_start(out=xq[ci][:], in_=xp_ap(ci))
        for d in range(ND):
            nc.sync.dma_start(out=wgw["b"][d][:],
                              in_=wts["b"]["wg"][d * P:(d + 1) * P, :])
        for ci in cb:
            nc.sync.dma_start(out=xq[ci][:], in_=xp_ap(ci))
        for j in (0, 1):
            nc.sync.dma_start(out=wot["b"][j][:], in_=wo_ap("b", j))
        # --- gpsimd queue: the other half of each Wo set; nothing late, so
        # its exit drain is trivial.
        for j in (2, 3):
            nc.gpsimd.dma_start(out=wot["a"][j][:], in_=wo_ap("a", j))
        for j in (2, 3):
            nc.gpsimd.dma_start(out=wot["b"][j][:], in_=wo_ap("b", j))

        # scalar-queue 4-DMA batches injected between gelus (the engine has
        # free completion sems and idle slots there): segment-B wv halves.
        def wv_batch(f):
            def go():
                for d in range(ND):
                    nc.scalar.dma_start(
                        out=wvh["b"][d][f][:],
                        in_=wts["b"]["wv"][d * P:(d + 1) * P,
                                           f * HW_:(f + 1) * HW_])
            return go

        inject = {}
        c_second = ca[1] if len(ca) > 1 else ca[0]
        inject[(c_second, 3)] = wv_batch(0)
        inject[(c_second, 11)] = wv_batch(1)

        # --- compute; stage B is emitted one chunk late so the PE never
        # waits on the ACT/DVE of the chunk it just produced.
        hT_of = {}

        def emit_A(ci, h_lo=0, h_hi=NH):
            tok0, mt, s = chunks[ci]
            if ci in hT_of:
                hT = hT_of[ci]
            else:
                hT = hpool.tile([P, NH, mt], BF, tag="hT", name="hT")
                hT_of[ci] = hT
            for h in range(h_lo, h_hi):
                pg = psg.tile([P, mt], f32, tag="pg", name="pg")
                pv = psv.tile([P, mt], f32, tag="pv", name="pv")
                for d in range(ND):
                    nc.tensor.matmul(out=pg[:], lhsT=wg_ap(s, d, h),
                                     rhs=xq[ci][:, d, :],
                                     start=(d == 0), stop=(d == ND - 1))
                for d in range(ND):
                    nc.tensor.matmul(out=pv[:], lhsT=wv_ap(s, d, h),
                                     rhs=xq[ci][:, d, :],
                                     start=(d == 0), stop=(d == ND - 1))
                ga = gpool.tile([P, mt], f32, tag="ga", name="ga")
                nc.scalar.activation(ga[:], pg[:], gelu)
                if (ci, h) in inject:
                    inject.pop((ci, h))()
                nc.vector.tensor_mul(hT[:, h, :], ga[:], pv[:])

        def emit_B(ci, last=False):
            tok0, mt, s = chunks[ci]
            hT = hT_of.pop(ci)
            wo_t = wot[s]
            for t in range(mt // P):
                po = pso.tile([P, D], f32, tag="po", name="po")
                for h in range(NH):
                    nc.tensor.matmul(
                        out=po[:], lhsT=hT[:, h, t * P:(t + 1) * P],
                        rhs=wo_t[h // 4][:, (h % 4) * D:(h % 4 + 1) * D],
                        start=(h == 0), stop=(h == NH - 1))
                ob = opool.tile([P, D], BF, tag="ob", name="ob")
                nc.vector.tensor_scalar_mul(ob[:], po[:], 1.0)
                j = tok0 // P + t
                nc.sync.dma_start(out=out[j * P:(j + 1) * P, :], in_=ob[:])

        prev = None
        for ci in range(len(chunks)):
            emit_A(ci)
            if prev is not None:
                emit_B(prev)
            prev = ci
        emit_B(prev, last=True)

        for go in list(inject.values()):
            go()

    nc.compile()
    return nc


def _pack(tiles, n_cores=8):
    """Bin-pack per-expert tile counts into n_cores cores x 2 segments.

    Returns (T, F, slots): each core has segment A capacity F tiles and
    segment B capacity T-F; slots is a list of
    (core, seg, expert, tile_lo, ntiles) with each (core, seg) single-expert.
    """
    total = sum(tiles)
    lo = max(1, -(-total // n_cores))
    for T in range(lo, max(tiles) + 1):
        F = (T + 1) // 2
        bins = [(F, c, "a") for c in range(n_cores)] + [
            (T - F, c, "b") for c in range(n_cores)]
        bins.sort(key=lambda b: -b[0])
        free = list(bins)
        slots = []
        ok = True
        for e in sorted(range(len(tiles)), key=lambda e: -tiles[e]):
            rem = tiles[e]
            lo_t = 0
            while rem > 0:
                if not free:
                    ok = False
                    break
                cap, c, seg = free.pop(0)
                take = min(cap, rem)
                slots.append((c, seg, e, lo_t, take))
                lo_t += take
                rem -= take
            if not ok:
                break
        if ok:
            return T, F, slots
    raise RuntimeError("packing failed")


def kernel(tokens, dispatch_weights, combine_weights, Wg, Wv, Wo, scales):
    from concourse.bass_utils import run_bass_kernel_spmd

    B, N, d_model = tokens.shape
    M = B * N
    x = np.ascontiguousarray(tokens.reshape(M, d_model), dtype=np.float32)
    disp = np.asarray(dispatch_weights).reshape(M, E)
    comb = np.asarray(combine_weights).reshape(M, E)
    w_all = np.where(disp > 0, comb, 0.0).astype(np.float32) * np.asarray(
        scales, np.float32)[None, :]

    idx = [np.nonzero(w_all[:, e])[0] for e in range(E)]
    tiles = [max(1, (len(i) + P - 1) // P) for i in idx]
    T, F, slots = _pack(tiles)
    NT, NTA = T * P, F * P

    if (NT, NTA) not in _CACHE:
        _CACHE[(NT, NTA)] = _build_program(NT, NTA)
    nc = _CACHE[(NT, NTA)]

    chunks = []
    t0 = 0
    for mt in _chunk_sizes(NTA):
        chunks.append((t0, mt))
        t0 += mt
    for mt in _chunk_sizes(NT - NTA):
        chunks.append((t0, mt))
        t0 += mt

    seg_of = {}
    for c, seg, e, lo_t, ntiles in slots:
        ids = idx[e][lo_t * P: lo_t * P + ntiles * P]
        seg_of[(c, seg)] = (e, ids)

    xT = x.T  # (D, M)
    wb = {}  # expert -> bf16 weight dict

    def expert_w(e):
        if e not in wb:
            woP = np.ascontiguousarray(
                np.asarray(Wo[e], np.float32).reshape(NH // 4, 4, P, D)
                .transpose(0, 2, 1, 3)).astype(BF16)
            wb[e] = {
                "wg": np.asarray(Wg[e], np.float32).astype(BF16),
                "wv": np.asarray(Wv[e], np.float32).astype(BF16),
                "wo": woP.reshape(-1),
            }
        return wb[e]

    in_maps = []
    for c in range(E):
        xTc = np.zeros((D, NT), np.float32)
        for seg, base in (("a", 0), ("b", NTA)):
            if (c, seg) in seg_of:
                e, ids = seg_of[(c, seg)]
                if len(ids):
                    xTc[:, base:base + len(ids)] = xT[:, ids]
        xp_c = np.empty(D * NT, BF16)
        off = 0
        for (tok0, mt) in chunks:
            blk = xTc[:, tok0:tok0 + mt].reshape(ND, P, mt).transpose(1, 0, 2)
            xp_c[off: off + P * ND * mt] = blk.astype(BF16).reshape(-1)
            off += P * ND * mt
        ea = seg_of.get((c, "a"), (0, None))[0]
        eb = seg_of.get((c, "b"), (ea, None))[0]
        wa, wbm = expert_w(ea), expert_w(eb)
        in_maps.append({
            "xp": xp_c,
            "wg_a": wa["wg"], "wv_a": wa["wv"], "wo_a": wa["wo"],
            "wg_b": wbm["wg"], "wv_b": wbm["wv"], "wo_b": wbm["wo"],
        })

    res = run_bass_kernel_spmd(nc, in_maps, list(range(E)))

    out = np.zeros((M, d_model), np.float32)
    for c in range(E):
        o = np.asarray(res.results[c]["out"]).astype(np.float32)
        for seg, base in (("a", 0), ("b", NTA)):
            if (c, seg) in seg_of:
                e, ids = seg_of[(c, seg)]
                if len(ids):
                    out[ids] += o[base:base + len(ids)] * w_all[ids, e][:, None]
    return out.reshape(B, N, d_model)


# revision 16
# speedup vs baseline: 1.1824x; 1.1824x over previous
"""Trainium2 Bass kernel for the AllGroupsExpertRunner MoE problem.

Math (dense-masked reference):
    x = tokens.reshape(M, D)                                # M = B*N = 8192
    out = sum_e w[:, e] * (gelu(x @ Wg[e]) * (x @ Wv[e])) @ Wo[e] * scales[e]
    where w = where(dispatch > 0, combine, 0)

Only tokens with w[:, e] > 0 matter for expert e. Sharding: two-segment
expert parallelism. Each core runs the same SPMD program over NT = T*128
token slots split at a fixed boundary NTA = F*128 into segments A and B,
each with its own full weight set. The host bin-packs the experts'
routed-token tiles into the 16 (core, segment) slots, so heavy experts
spill across cores and the per-core token count is ~total/8 instead of
max-per-expert (2176 vs 2304 for the reference routing). Routing weights
and output scales are applied on the host during the scatter-add.

All matmul operands are bf16 (fp32 PSUM accumulation): 1 row/cycle on the
PE, half the HBM traffic of fp32. Measured rel err ~4e-3 vs the 2e-2 gate.

DMA scheduling notes (from trace analysis): each engine has only 4 DMA
completion semaphores, so the 5th dma_start on an engine blocks until the
1st fully completes. The scalar engine must run gelus from ~t+15us, so it
gets at most 4 up-front DMAs; its later weight loads are injected between
gelus where the completion waits are free. Queue FIFO delivery (~200GB/s)
paces the start, so the first-needed wgA tiles are quarter tiles right
behind x chunk 0 on the sync queue, and wvA h-half tiles load on the
scalar queue in parallel. Wo / late x / output tiles ride the gpsimd
SWDGE queue (~70GB/s), which has no compute to block.

Per-core program, per token chunk (<=512):
  stage A, per 128-wide H block:
      g^T = Wg_blk^T @ xT-chunk   (4 accumulating matmuls over D)
      v^T = Wv_blk^T @ xT-chunk
      hT_blk = gelu(g^T) * v^T    (ACT + DVE, bf16 out)
  stage B (emitted one chunk late so hT is long since ready), per
  128-token tile:
      out_tile = hT^T @ Wo        (16 accumulating matmuls over H)
      DVE copy PSUM -> bf16, DMA out on the gpsimd queue.
"""

import numpy as np
import ml_dtypes

D = 512
H = 2048
E = 8
P = 128
MT = 512  # max token chunk (PSUM bank = 512 fp32)
ND = D // P  # 4 k-tiles over D
NH = H // P  # 16 k-tiles over H

_CACHE: dict = {}

BF16 = ml_dtypes.bfloat16


def _chunk_sizes(n):
    out = [MT] * (n // MT)
    if n % MT:
        out.append(n % MT)
    return out


def _build_program(NT: int, NTA: int):
    from contextlib import ExitStack

    import concourse.bacc as bacc
    import concourse.tile as tile
    import concourse.mybir as mybir
    import concourse.bass as bass_mod

    assert NT % P == 0 and NTA % P == 0 and 0 < NTA <= NT
    f32 = mybir.dt.float32
    BF = mybir.dt.bfloat16

    nc = bacc.Bacc("TRN2", target_bir_lowering=False, debug=False)

    xp = nc.dram_tensor("xp", [D * NT], BF, kind="ExternalInput")
    wts = {}
    for s in ("a", "b"):
        wts[s] = {
            "wg": nc.dram_tensor(f"wg_{s}", [D, H], BF, kind="ExternalInput"),
            "wv": nc.dram_tensor(f"wv_{s}", [D, H], BF, kind="ExternalInput"),
            # woP: host-packed [NH//4, 128, 4*D]: tile j row p holds
            # wo[(4j+k)*128+p, :] for k=0..3 -> 4KB DMA rows
            "wo": nc.dram_tensor(f"wo_{s}", [NH // 4 * P * 4 * D], BF,
                                 kind="ExternalInput"),
        }
    out = nc.dram_tensor("out", [NT, D], BF, kind="ExternalOutput")

    # chunk list: (token_offset, size, segment)
    chunks = []
    t0 = 0
    for mt in _chunk_sizes(NTA):
        chunks.append((t0, mt, "a"))
        t0 += mt
    for mt in _chunk_sizes(NT - NTA):
        chunks.append((t0, mt, "b"))
        t0 += mt

    gelu = mybir.ActivationFunctionType.Gelu

    with tile.TileContext(nc) as tc, ExitStack() as ctx:
        wpool = ctx.enter_context(tc.tile_pool(name="w", bufs=1))
        xpool = ctx.enter_context(tc.tile_pool(name="x", bufs=1))
        hpool = ctx.enter_context(tc.tile_pool(name="h", bufs=3))
        gpool = ctx.enter_context(tc.tile_pool(name="g", bufs=4))
        opool = ctx.enter_context(tc.tile_pool(name="o", bufs=6))
        psg = ctx.enter_context(tc.tile_pool(name="psg", bufs=3, space="PSUM"))
        psv = ctx.enter_context(tc.tile_pool(name="psv", bufs=3, space="PSUM"))
        pso = ctx.enter_context(tc.tile_pool(name="pso", bufs=2, space="PSUM"))

        # weight tiles: wg segment A in 512-col quarters (one DMA each, so
        # the first matmul only waits on x + one 128KB tile), wg segment B
        # whole [128, 2048] tiles, wv both segments in 1024-col halves
        # (loaded on the scalar queue in 4-DMA batches), wo in 4-hslice
        # merged tiles (host-packed for 4KB DMA rows).
        QW, HW_ = 512, 1024
        wgq = {"a": [[wpool.tile([P, QW], BF, tag=f"wga{d}q{q}", name=f"wga{d}q{q}")
                      for q in range(H // QW)] for d in range(ND)]}
        wgw = {"b": [wpool.tile([P, H], BF, tag=f"wgb{d}", name=f"wgb{d}")
                     for d in range(ND)]}
        wvq = {"a": [[wpool.tile([P, QW], BF, tag=f"wva{d}q{q}", name=f"wva{d}q{q}")
                      for q in range(H // QW)] for d in range(ND)]}
        wvh = {"b": [[wpool.tile([P, HW_], BF, tag=f"wvb{d}f{f}", name=f"wvb{d}f{f}")
                      for f in range(H // HW_)] for d in range(ND)]}
        wot = {s: [wpool.tile([P, 4 * D], BF, tag=f"wo{s}{j}", name=f"wo{s}{j}")
                   for j in range(NH // 4)] for s in ("a", "b")}

        def wg_ap(s, d, h):
            if s == "a":
                q, c = divmod(h * P, QW)
                return wgq["a"][d][q][:, c:c + P]
            return wgw["b"][d][:, h * P:(h + 1) * P]

        def wv_ap(s, d, h):
            if s == "a":
                q, c = divmod(h * P, QW)
                return wvq["a"][d][q][:, c:c + P]
            f, c = divmod(h * P, HW_)
            return wvh["b"][d][f][:, c:c + P]

        # x chunk tiles: [128, ND, mt], host-packed so row p holds
        # xT[d*128+p, tok0:tok0+mt] for d=0..3 (4KB rows at mt=512)
        xq = []
        xoff = []
        off = 0
        for (tok0, mt, s) in chunks:
            xq.append(xpool.tile([P, ND, mt], BF, tag=f"xq{tok0}",
                                 name=f"xq{tok0}"))
            xoff.append(off)
            off += P * ND * mt

        def xp_ap(ci):
            tok0, mt, _ = chunks[ci]
            return bass_mod.AP(tensor=xp, offset=xoff[ci],
                               ap=[[ND * mt, P], [1, ND * mt]])

        def wo_ap(s, j):
            return bass_mod.AP(tensor=wts[s]["wo"], offset=j * P * 4 * D,
                               ap=[[4 * D, P], [1, 4 * D]])

        ca = [ci for ci, c in enumerate(chunks) if c[2] == "a"]
        cb = [ci for ci, c in enumerate(chunks) if c[2] == "b"]

        # act-table preload: a dummy gelu on a zeroed scratch tile makes the
        # scalar engine pull the gelu table during the DMA warmup instead of
        # right before the first real gelu.
        # scalar queue: exactly its 4 free up-front DMA slots carry wv q0 in
        # parallel with sync's x+wg q0, pulling the first v-matmul earlier.
        for d in range(ND):
            nc.scalar.dma_start(
                out=wvq["a"][d][0][:],
                in_=wts["a"]["wv"][d * P:(d + 1) * P, 0:QW])
        scratch = gpool.tile([P, 8], f32, tag="scratch", name="scratch")
        nc.vector.memset(scratch[:], 0.0)
        nc.scalar.activation(scratch[:], scratch[:], gelu)

        # --- sync queue, strict demand order: x chunk 0 whole (4KB rows run
        # at full packet rate even on the cold queue), then wg/wv quarter
        # tiles interleaved in h-consumption order, then segment-A Wo, later
        # x chunks, and the segment-B bulk. Output DMAs are appended by
        # emit_B behind these. Keeping the critical stream on one queue in
        # exact consumption order measured better than spreading it across
        # the cold scalar/gpsimd queues.
        nc.sync.dma_start(out=xq[ca[0]][:], in_=xp_ap(ca[0]))
        for q in range(H // QW):
            for d in range(ND):
                nc.sync.dma_start(
                    out=wgq["a"][d][q][:],
                    in_=wts["a"]["wg"][d * P:(d + 1) * P, q * QW:(q + 1) * QW])
            if q == 0:
                continue  # wv q0 rides the scalar queue in parallel
            for d in range(ND):
                nc.sync.dma_start(
                    out=wvq["a"][d][q][:],
                    in_=wts["a"]["wv"][d * P:(d + 1) * P, q * QW:(q + 1) * QW])
        for j in (0, 1):
            nc.sync.dma_start(out=wot["a"][j][:], in_=wo_ap("a", j))
        for ci in ca[1:]:
            nc.sync.dma_start(out=xq[ci][:], in_=xp_ap(ci))
        for d in range(ND):
            nc.sync.dma_start(out=wgw["b"][d][:],
                              in_=wts["b"]["wg"][d * P:(d + 1) * P, :])
        for ci in cb:
            nc.sync.dma_start(out=xq[ci][:], in_=xp_ap(ci))
        for j in (0, 1):
            nc.sync.dma_start(out=wot["b"][j][:], in_=wo_ap("b", j))
        # --- gpsimd queue: the other half of each Wo set; nothing late, so
        # its exit drain is trivial.
        for j in (2, 3):
            nc.gpsimd.dma_start(out=wot["a"][j][:], in_=wo_ap("a", j))
        for j in (2, 3):
            nc.gpsimd.dma_start(out=wot["b"][j][:], in_=wo_ap("b", j))

        # scalar-queue 4-DMA batches injected between gelus (the engine has
        # free completion sems and idle slots there): segment-B wv halves.
        def wv_batch(f):
            def go():
                for d in range(ND):
                    nc.scalar.dma_start(
                        out=wvh["b"][d][f][:],
                        in_=wts["b"]["wv"][d * P:(d + 1) * P,
                                           f * HW_:(f + 1) * HW_])
            return go

        inject = {}
        c_second = ca[1] if len(ca) > 1 else ca[0]
        inject[(c_second, 3)] = wv_batch(0)
        inject[(c_second, 11)] = wv_batch(1)

        # --- compute; stage B is emitted one chunk late so the PE never
        # waits on the ACT/DVE of the chunk it just produced.
        hT_of = {}

        def emit_A(ci, h_lo=0, h_hi=NH):
            tok0, mt, s = chunks[ci]
            if ci in hT_of:
                hT = hT_of[ci]
            else:
                hT = hpool.tile([P, NH, mt], BF, tag="hT", name="hT")
                hT_of[ci] = hT
            for h in range(h_lo, h_hi):
                pg = psg.tile([P, mt], f32, tag="pg", name="pg")
                pv = psv.tile([P, mt], f32, tag="pv", name="pv")
                for d in range(ND):
                    nc.tensor.matmul(out=pg[:], lhsT=wg_ap(s, d, h),
                                     rhs=xq[ci][:, d, :],
                                     start=(d == 0), stop=(d == ND - 1))
                for d in range(ND):
                    nc.tensor.matmul(out=pv[:], lhsT=wv_ap(s, d, h),
                                     rhs=xq[ci][:, d, :],
                                     start=(d == 0), stop=(d == ND - 1))
                ga = gpool.tile([P, mt], f32, tag="ga", name="ga")
                nc.scalar.activation(ga[:], pg[:], gelu)
                if (ci, h) in inject:
                    inject.pop((ci, h))()
                nc.vector.tensor_mul(hT[:, h, :], ga[:], pv[:])

        def emit_B(ci, last=False):
            tok0, mt, s = chunks[ci]
            hT = hT_of.pop(ci)
            wo_t = wot[s]
            for t in range(mt // P):
                po = pso.tile([P, D], f32, tag="po", name="po")
                for h in range(NH):
                    nc.tensor.matmul(
                        out=po[:], lhsT=hT[:, h, t * P:(t + 1) * P],
                        rhs=wo_t[h // 4][:, (h % 4) * D:(h % 4 + 1) * D],
                        start=(h == 0), stop=(h == NH - 1))
                ob = opool.tile([P, D], BF, tag="ob", name="ob")
                nc.vector.tensor_scalar_mul(ob[:], po[:], 1.0)
                j = tok0 // P + t
                nc.sync.dma_start(out=out[j * P:(j + 1) * P, :], in_=ob[:])

        # process the small (non-512) chunk last: its stage B has a single
        # output tile, so the post-last-matmul tail (copy + out DMA before
        # the teardown barriers) is as short as possible.
        order = list(range(len(chunks)))
        small = [ci for ci in order if chunks[ci][1] != MT]
        if small and len(order) > 1:
            sm = small[0]
            order = [ci for ci in order if ci != sm] + [sm]
        prev = None
        for ci in order:
            emit_A(ci)
            if prev is not None:
                emit_B(prev)
            prev = ci
        emit_B(prev, last=True)

        for go in list(inject.values()):
            go()

    nc.compile()
    return nc


def _pack(tiles, n_cores=8):
    """Bin-pack per-expert tile counts into n_cores cores x 2 segments.

    Returns (T, F, slots): each core has segment A capacity F tiles and
    segment B capacity T-F; slots is a list of
    (core, seg, expert, tile_lo, ntiles) with each (core, seg) single-expert.
    """
    total = sum(tiles)
    lo = max(1, -(-total // n_cores))
    for T in range(lo, max(tiles) + 1):
        F = (T + 1) // 2
        bins = [(F, c, "a") for c in range(n_cores)] + [
            (T - F, c, "b") for c in range(n_cores)]
        bins.sort(key=lambda b: -b[0])
        free = list(bins)
        slots = []
        ok = True
        for e in sorted(range(len(tiles)), key=lambda e: -tiles[e]):
            rem = tiles[e]
            lo_t = 0
            while rem > 0:
                if not free:
                    ok = False
                    break
                cap, c, seg = free.pop(0)
                take = min(cap, rem)
                slots.append((c, seg, e, lo_t, take))
                lo_t += take
                rem -= take
            if not ok:
                break
        if ok:
            return T, F, slots
    raise RuntimeError("packing failed")


def kernel(tokens, dispatch_weights, combine_weights, Wg, Wv, Wo, scales):
    from concourse.bass_utils import run_bass_kernel_spmd

    B, N, d_model = tokens.shape
    M = B * N
    x = np.ascontiguousarray(tokens.reshape(M, d_model), dtype=np.float32)
    disp = np.asarray(dispatch_weights).reshape(M, E)
    comb = np.asarray(combine_weights).reshape(M, E)
    w_all = np.where(disp > 0, comb, 0.0).astype(np.float32) * np.asarray(
        scales, np.float32)[None, :]

    idx = [np.nonzero(w_all[:, e])[0] for e in range(E)]
    tiles = [max(1, (len(i) + P - 1) // P) for i in idx]
    T, F, slots = _pack(tiles)
    NT, NTA = T * P, F * P

    if (NT, NTA) not in _CACHE:
        _CACHE[(NT, NTA)] = _build_program(NT, NTA)
    nc = _CACHE[(NT, NTA)]

    chunks = []
    t0 = 0
    for mt in _chunk_sizes(NTA):
        chunks.append((t0, mt))
        t0 += mt
    for mt in _chunk_sizes(NT - NTA):
        chunks.append((t0, mt))
        t0 += mt

    seg_of = {}
    for c, seg, e, lo_t, ntiles in slots:
        ids = idx[e][lo_t * P: lo_t * P + ntiles * P]
        seg_of[(c, seg)] = (e, ids)

    xT = x.T  # (D, M)
    wb = {}  # expert -> bf16 weight dict

    def expert_w(e):
        if e not in wb:
            woP = np.ascontiguousarray(
                np.asarray(Wo[e], np.float32).reshape(NH // 4, 4, P, D)
                .transpose(0, 2, 1, 3)).astype(BF16)
            wb[e] = {
                "wg": np.asarray(Wg[e], np.float32).astype(BF16),
                "wv": np.asarray(Wv[e], np.float32).astype(BF16),
                "wo": woP.reshape(-1),
            }
        return wb[e]

    in_maps = []
    for c in range(E):
        xTc = np.zeros((D, NT), np.float32)
        for seg, base in (("a", 0), ("b", NTA)):
            if (c, seg) in seg_of:
                e, ids = seg_of[(c, seg)]
                if len(ids):
                    xTc[:, base:base + len(ids)] = xT[:, ids]
        xp_c = np.empty(D * NT, BF16)
        off = 0
        for (tok0, mt) in chunks:
            blk = xTc[:, tok0:tok0 + mt].reshape(ND, P, mt).transpose(1, 0, 2)
            xp_c[off: off + P * ND * mt] = blk.astype(BF16).reshape(-1)
            off += P * ND * mt
        ea = seg_of.get((c, "a"), (0, None))[0]
        eb = seg_of.get((c, "b"), (ea, None))[0]
        wa, wbm = expert_w(ea), expert_w(eb)
        in_maps.append({
            "xp": xp_c,
            "wg_a": wa["wg"], "wv_a": wa["wv"], "wo_a": wa["wo"],
            "wg_b": wbm["wg"], "wv_b": wbm["wv"], "wo_b": wbm["wo"],
        })

    res = run_bass_kernel_spmd(nc, in_maps, list(range(E)))

    out = np.zeros((M, d_model), np.float32)
    for c in range(E):
        o = np.asarray(res.results[c]["out"]).astype(np.float32)
        for seg, base in (("a", 0), ("b", NTA)):
            if (c, seg) in seg_of:
                e, ids = seg_of[(c, seg)]
                if len(ids):
                    out[ids] += o[base:base + len(ids)] * w_all[ids, e][:, None]
    return out.reshape(B, N, d_model)


# revision 18
# speedup vs baseline: 1.1888x; 1.0054x over previous
"""Trainium2 Bass kernel for the AllGroupsExpertRunner MoE problem.

Math (dense-masked reference):
    x = tokens.reshape(M, D)                                # M = B*N = 8192
    out = sum_e w[:, e] * (gelu(x @ Wg[e]) * (x @ Wv[e])) @ Wo[e] * scales[e]
    where w = where(dispatch > 0, combine, 0)

Only tokens with w[:, e] > 0 matter for expert e. Sharding: two-segment
expert parallelism. Each core runs the same SPMD program over NT = T*128
token slots split at a fixed boundary NTA = F*128 into segments A and B,
each with its own full weight set. The host bin-packs the experts'
routed-token tiles into the 16 (core, segment) slots, so heavy experts
spill across cores and the per-core token count is ~total/8 instead of
max-per-expert (2176 vs 2304 for the reference routing). Routing weights
and output scales are applied on the host during the scatter-add.

All matmul operands are bf16 (fp32 PSUM accumulation): 1 row/cycle on the
PE, half the HBM traffic of fp32. Measured rel err ~4e-3 vs the 2e-2 gate.

DMA scheduling notes (from trace analysis): each engine has only 4 DMA
completion semaphores, so the 5th dma_start on an engine blocks until the
1st fully completes. The scalar engine must run gelus from ~t+15us, so it
gets at most 4 up-front DMAs; its later weight loads are injected between
gelus where the completion waits are free. Queue FIFO delivery (~200GB/s)
paces the start, so the first-needed wgA tiles are quarter tiles right
behind x chunk 0 on the sync queue, and wvA h-half tiles load on the
scalar queue in parallel. Wo / late x / output tiles ride the gpsimd
SWDGE queue (~70GB/s), which has no compute to block.

Per-core program, per token chunk (<=512):
  stage A, per 128-wide H block:
      g^T = Wg_blk^T @ xT-chunk   (4 accumulating matmuls over D)
      v^T = Wv_blk^T @ xT-chunk
      hT_blk = gelu(g^T) * v^T    (ACT + DVE, bf16 out)
  stage B (emitted one chunk late so hT is long since ready), per
  128-token tile:
      out_tile = hT^T @ Wo        (16 accumulating matmuls over H)
      DVE copy PSUM -> bf16, DMA out on the gpsimd queue.
"""

import numpy as np
import ml_dtypes

D = 512
H = 2048
E = 8
P = 128
MT = 512  # max token chunk (PSUM bank = 512 fp32)
ND = D // P  # 4 k-tiles over D
NH = H // P  # 16 k-tiles over H

_CACHE: dict = {}

BF16 = ml_dtypes.bfloat16


def _chunk_sizes(n):
    out = [MT] * (n // MT)
    if n % MT:
        out.append(n % MT)
    return out


def _build_program(NT: int, NTA: int):
    from contextlib import ExitStack

    import concourse.bacc as bacc
    import concourse.tile as tile
    import concourse.mybir as mybir
    import concourse.bass as bass_mod

    assert NT % P == 0 and NTA % P == 0 and 0 < NTA <= NT
    f32 = mybir.dt.float32
    BF = mybir.dt.bfloat16

    nc = bacc.Bacc("TRN2", target_bir_lowering=False, debug=False)

    xp = nc.dram_tensor("xp", [D * NT], BF, kind="ExternalInput")
    wts = {}
    for s in ("a", "b"):
        wts[s] = {
            "wg": nc.dram_tensor(f"wg_{s}", [D, H], BF, kind="ExternalInput"),
            "wv": nc.dram_tensor(f"wv_{s}", [D, H], BF, kind="ExternalInput"),
            # woP: host-packed [NH//4, 128, 4*D]: tile j row p holds
            # wo[(4j+k)*128+p, :] for k=0..3 -> 4KB DMA rows
            "wo": nc.dram_tensor(f"wo_{s}", [NH // 4 * P * 4 * D], BF,
                                 kind="ExternalInput"),
        }
    out = nc.dram_tensor("out", [NT, D], BF, kind="ExternalOutput")

    # chunk list: (token_offset, size, segment)
    chunks = []
    t0 = 0
    for mt in _chunk_sizes(NTA):
        chunks.append((t0, mt, "a"))
        t0 += mt
    for mt in _chunk_sizes(NT - NTA):
        chunks.append((t0, mt, "b"))
        t0 += mt

    gelu = mybir.ActivationFunctionType.Gelu

    with tile.TileContext(nc) as tc, ExitStack() as ctx:
        wpool = ctx.enter_context(tc.tile_pool(name="w", bufs=1))
        xpool = ctx.enter_context(tc.tile_pool(name="x", bufs=1))
        hpool = ctx.enter_context(tc.tile_pool(name="h", bufs=3))
        gpool = ctx.enter_context(tc.tile_pool(name="g", bufs=4))
        opool = ctx.enter_context(tc.tile_pool(name="o", bufs=6))
        psg = ctx.enter_context(tc.tile_pool(name="psg", bufs=3, space="PSUM"))
        psv = ctx.enter_context(tc.tile_pool(name="psv", bufs=3, space="PSUM"))
        pso = ctx.enter_context(tc.tile_pool(name="pso", bufs=2, space="PSUM"))

        # weight tiles: wg segment A in 512-col quarters (one DMA each, so
        # the first matmul only waits on x + one 128KB tile), wg segment B
        # whole [128, 2048] tiles, wv both segments in 1024-col halves
        # (loaded on the scalar queue in 4-DMA batches), wo in 4-hslice
        # merged tiles (host-packed for 4KB DMA rows).
        QW, HW_ = 512, 1024
        wgq = {"a": [[wpool.tile([P, QW], BF, tag=f"wga{d}q{q}", name=f"wga{d}q{q}")
                      for q in range(H // QW)] for d in range(ND)]}
        wgw = {"b": [wpool.tile([P, H], BF, tag=f"wgb{d}", name=f"wgb{d}")
                     for d in range(ND)]}
        wvq = {"a": [[wpool.tile([P, QW], BF, tag=f"wva{d}q{q}", name=f"wva{d}q{q}")
                      for q in range(H // QW)] for d in range(ND)]}
        wvh = {"b": [[wpool.tile([P, HW_], BF, tag=f"wvb{d}f{f}", name=f"wvb{d}f{f}")
                      for f in range(H // HW_)] for d in range(ND)]}
        wot = {s: [wpool.tile([P, 4 * D], BF, tag=f"wo{s}{j}", name=f"wo{s}{j}")
                   for j in range(NH // 4)] for s in ("a", "b")}

        def wg_ap(s, d, h):
            if s == "a":
                q, c = divmod(h * P, QW)
                return wgq["a"][d][q][:, c:c + P]
            return wgw["b"][d][:, h * P:(h + 1) * P]

        def wv_ap(s, d, h):
            if s == "a":
                q, c = divmod(h * P, QW)
                return wvq["a"][d][q][:, c:c + P]
            f, c = divmod(h * P, HW_)
            return wvh["b"][d][f][:, c:c + P]

        # x chunk tiles: [128, ND, mt], host-packed so row p holds
        # xT[d*128+p, tok0:tok0+mt] for d=0..3 (4KB rows at mt=512)
        xq = []
        xoff = []
        off = 0
        for (tok0, mt, s) in chunks:
            xq.append(xpool.tile([P, ND, mt], BF, tag=f"xq{tok0}",
                                 name=f"xq{tok0}"))
            xoff.append(off)
            off += P * ND * mt

        def xp_ap(ci):
            tok0, mt, _ = chunks[ci]
            return bass_mod.AP(tensor=xp, offset=xoff[ci],
                               ap=[[ND * mt, P], [1, ND * mt]])

        def wo_ap(s, j):
            return bass_mod.AP(tensor=wts[s]["wo"], offset=j * P * 4 * D,
                               ap=[[4 * D, P], [1, 4 * D]])

        ca = [ci for ci, c in enumerate(chunks) if c[2] == "a"]
        cb = [ci for ci, c in enumerate(chunks) if c[2] == "b"]

        # act-table preload: a dummy gelu on a zeroed scratch tile makes the
        # scalar engine pull the gelu table during the DMA warmup instead of
        # right before the first real gelu.
        # scalar queue: exactly its 4 free up-front DMA slots carry wv q0 in
        # parallel with sync's x+wg q0, pulling the first v-matmul earlier.
        for d in range(ND):
            nc.scalar.dma_start(
                out=wvq["a"][d][0][:],
                in_=wts["a"]["wv"][d * P:(d + 1) * P, 0:QW])
        scratch = gpool.tile([P, 8], f32, tag="scratch", name="scratch")
        nc.vector.memset(scratch[:], 0.0)
        nc.scalar.activation(scratch[:], scratch[:], gelu)

        # --- sync queue, strict demand order: x chunk 0 whole (4KB rows run
        # at full packet rate even on the cold queue), then wg/wv quarter
        # tiles interleaved in h-consumption order, then segment-A Wo, later
        # x chunks, and the segment-B bulk. Output DMAs are appended by
        # emit_B behind these. Keeping the critical stream on one queue in
        # exact consumption order measured better than spreading it across
        # the cold scalar/gpsimd queues.
        nc.sync.dma_start(out=xq[ca[0]][:], in_=xp_ap(ca[0]))
        for q in range(H // QW):
            for d in range(ND):
                nc.sync.dma_start(
                    out=wgq["a"][d][q][:],
                    in_=wts["a"]["wg"][d * P:(d + 1) * P, q * QW:(q + 1) * QW])
            if q == 0:
                continue  # wv q0 rides the scalar queue in parallel
            for d in range(ND):
                nc.sync.dma_start(
                    out=wvq["a"][d][q][:],
                    in_=wts["a"]["wv"][d * P:(d + 1) * P, q * QW:(q + 1) * QW])
        for j in (0, 1):
            nc.sync.dma_start(out=wot["a"][j][:], in_=wo_ap("a", j))
        for ci in ca[1:]:
            nc.sync.dma_start(out=xq[ci][:], in_=xp_ap(ci))
        for d in range(ND):
            nc.sync.dma_start(out=wgw["b"][d][:],
                              in_=wts["b"]["wg"][d * P:(d + 1) * P, :])
        for ci in cb:
            nc.sync.dma_start(out=xq[ci][:], in_=xp_ap(ci))
        for j in (0, 1):
            nc.sync.dma_start(out=wot["b"][j][:], in_=wo_ap("b", j))
        # --- gpsimd queue: the other half of each Wo set; nothing late, so
        # its exit drain is trivial.
        for j in (2, 3):
            nc.gpsimd.dma_start(out=wot["a"][j][:], in_=wo_ap("a", j))
        for j in (2, 3):
            nc.gpsimd.dma_start(out=wot["b"][j][:], in_=wo_ap("b", j))

        # scalar-queue 4-DMA batches injected between gelus (the engine has
        # free completion sems and idle slots there): segment-B wv halves.
        def wv_batch(f):
            def go():
                for d in range(ND):
                    nc.scalar.dma_start(
                        out=wvh["b"][d][f][:],
                        in_=wts["b"]["wv"][d * P:(d + 1) * P,
                                           f * HW_:(f + 1) * HW_])
            return go

        inject = {}
        c_second = ca[1] if len(ca) > 1 else ca[0]
        inject[(c_second, 3)] = wv_batch(0)
        inject[(c_second, 11)] = wv_batch(1)

        # --- compute; stage B is emitted one chunk late so the PE never
        # waits on the ACT/DVE of the chunk it just produced.
        hT_of = {}

        def emit_A(ci, h_lo=0, h_hi=NH):
            tok0, mt, s = chunks[ci]
            if ci in hT_of:
                hT = hT_of[ci]
            else:
                hT = hpool.tile([P, NH, mt], BF, tag="hT", name="hT")
                hT_of[ci] = hT
            for h in range(h_lo, h_hi):
                pg = psg.tile([P, mt], f32, tag="pg", name="pg")
                pv = psv.tile([P, mt], f32, tag="pv", name="pv")
                for d in range(ND):
                    nc.tensor.matmul(out=pg[:], lhsT=wg_ap(s, d, h),
                                     rhs=xq[ci][:, d, :],
                                     start=(d == 0), stop=(d == ND - 1))
                for d in range(ND):
                    nc.tensor.matmul(out=pv[:], lhsT=wv_ap(s, d, h),
                                     rhs=xq[ci][:, d, :],
                                     start=(d == 0), stop=(d == ND - 1))
                ga = gpool.tile([P, mt], f32, tag="ga", name="ga")
                nc.scalar.activation(ga[:], pg[:], gelu)
                if (ci, h) in inject:
                    inject.pop((ci, h))()
                nc.vector.tensor_mul(hT[:, h, :], ga[:], pv[:])

        def emit_B(ci, last=False):
            tok0, mt, s = chunks[ci]
            hT = hT_of.pop(ci)
            wo_t = wot[s]
            for t in range(mt // P):
                po = pso.tile([P, D], f32, tag="po", name="po")
                for h in range(NH):
                    nc.tensor.matmul(
                        out=po[:], lhsT=hT[:, h, t * P:(t + 1) * P],
                        rhs=wo_t[h // 4][:, (h % 4) * D:(h % 4 + 1) * D],
                        start=(h == 0), stop=(h == NH - 1))
                ob = opool.tile([P, D], BF, tag="ob", name="ob")
                nc.vector.tensor_scalar_mul(ob[:], po[:], 1.0)
                j = tok0 // P + t
                nc.sync.dma_start(out=out[j * P:(j + 1) * P, :], in_=ob[:])

        # process the small (non-512) chunk last: its stage B has a single
        # output tile, so the post-last-matmul tail (copy + out DMA before
        # the teardown barriers) is as short as possible.
        order = list(range(len(chunks)))
        small = [ci for ci in order if chunks[ci][1] != MT]
        if small and len(order) > 1:
            sm = small[0]
            order = [ci for ci in order if ci != sm] + [sm]
        prev = None
        for ci in order:
            emit_A(ci)
            if prev is not None:
                emit_B(prev)
            prev = ci
        emit_B(prev, last=True)

        for go in list(inject.values()):
            go()

    nc.compile()
    return nc


def _pack(tiles, n_cores=8):
    """Bin-pack per-expert tile counts into n_cores cores x 2 segments.

    Returns (T, F, slots): each core has segment A capacity F tiles and
    segment B capacity T-F; slots is a list of
    (core, seg, expert, tile_lo, ntiles) with each (core, seg) single-expert.
    """
    total = sum(tiles)
    lo = max(1, -(-total // n_cores))
    for T in range(lo, max(tiles) + 1):
        F = (T + 1) // 2
        bins = [(F, c, "a") for c in range(n_cores)] + [
            (T - F, c, "b") for c in range(n_cores)]
        bins.sort(key=lambda b: -b[0])
        free = list(bins)
        slots = []
        ok = True
        for e in sorted(range(len(tiles)), key=lambda e: -tiles[e]):
            rem = tiles[e]
            lo_t = 0
            while rem > 0:
                if not free:
                    ok = False
                    break
                cap, c, seg = free.pop(0)
                take = min(cap, rem)
                slots.append((c, seg, e, lo_t, take))
                lo_t += take
                rem -= take
            if not ok:
                break
        if ok:
            return T, F, slots
    raise RuntimeError("packing failed")


def kernel(tokens, dispatch_weights, combine_weights, Wg, Wv, Wo, scales):
    from concourse.bass_utils import run_bass_kernel_spmd

    B, N, d_model = tokens.shape
    M = B * N
    x = np.ascontiguousarray(tokens.reshape(M, d_model), dtype=np.float32)
    disp = np.asarray(dispatch_weights).reshape(M, E)
    comb = np.asarray(combine_weights).reshape(M, E)
    w_all = np.where(disp > 0, comb, 0.0).astype(np.float32) * np.asarray(
        scales, np.float32)[None, :]

    idx = [np.nonzero(w_all[:, e])[0] for e in range(E)]
    tiles = [max(1, (len(i) + P - 1) // P) for i in idx]
    T, F, slots = _pack(tiles)
    NT, NTA = T * P, F * P

    if (NT, NTA) not in _CACHE:
        _CACHE[(NT, NTA)] = _build_program(NT, NTA)
    nc = _CACHE[(NT, NTA)]

    chunks = []
    t0 = 0
    for mt in _chunk_sizes(NTA):
        chunks.append((t0, mt))
        t0 += mt
    for mt in _chunk_sizes(NT - NTA):
        chunks.append((t0, mt))
        t0 += mt

    seg_of = {}
    for c, seg, e, lo_t, ntiles in slots:
        ids = idx[e][lo_t * P: lo_t * P + ntiles * P]
        seg_of[(c, seg)] = (e, ids)

    xT = x.T  # (D, M)
    wb = {}  # expert -> bf16 weight dict

    def expert_w(e):
        if e not in wb:
            woP = np.ascontiguousarray(
                np.asarray(Wo[e], np.float32).reshape(NH // 4, 4, P, D)
                .transpose(0, 2, 1, 3)).astype(BF16)
            wb[e] = {
                "wg": np.asarray(Wg[e], np.float32).astype(BF16),
                "wv": np.asarray(Wv[e], np.float32).astype(BF16),
                "wo": woP.reshape(-1),
            }
        return wb[e]

    in_maps = []
    for c in range(E):
        xTc = np.zeros((D, NT), np.float32)
        for seg, base in (("a", 0), ("b", NTA)):
            if (c, seg) in seg_of:
                e, ids = seg_of[(c, seg)]
                if len(ids):
                    xTc[:, base:base + len(ids)] = xT[:, ids]
        xp_c = np.empty(D * NT, BF16)
        off = 0
        for (tok0, mt) in chunks:
            blk = xTc[:, tok0:tok0 + mt].reshape(ND, P, mt).transpose(1, 0, 2)
            xp_c[off: off + P * ND * mt] = blk.astype(BF16).reshape(-1)
            off += P * ND * mt
        ea = seg_of.get((c, "a"), (0, None))[0]
        eb = seg_of.get((c, "b"), (ea, None))[0]
        wa, wbm = expert_w(ea), expert_w(eb)
        in_maps.append({
            "xp": xp_c,
            "wg_a": wa["wg"], "wv_a": wa["wv"], "wo_a": wa["wo"],
            "wg_b": wbm["wg"], "wv_b": wbm["wv"], "wo_b": wbm["wo"],
        })

    res = run_bass_kernel_spmd(nc, in_maps, list(range(E)))

    out = np.zeros((M, d_model), np.float32)
    for c in range(E):
        o = np.asarray(res.results[c]["out"]).astype(np.float32)
        for seg, base in (("a", 0), ("b", NTA)):
            if (c, seg) in seg_of:
                e, ids = seg_of[(c, seg)]
                if len(ids):
                    out[ids] += o[base:base + len(ids)] * w_all[ids, e][:, None]
    return out.reshape(B, N, d_model)
